# revision 28
# baseline (speedup 1.0000x reference)
"""Trainium2 Bass kernel for causal GQA attention (nn_Attention_83090437308676).

Full shapes: x [4096, 2048], 16 Q heads / 4 KV heads, d_head=128, fp32, causal,
rotary (interleaved pairs, rotary_dim=128), out = attn @ W_O + b_O.

Sharding: tensor-parallel over heads. Core c computes Q-heads {2c, 2c+1} and
KV-head c//2 (duplicated across the pair of cores sharing it), produces the
partial output z_h @ W_O_h summed over its 2 heads; the host sums the 8
partials and adds b_O.

Device-side layout trick: all matmuls contract on the partition axis, so x is
fed pre-transposed (xT [d_model, seq]) and Q/K are produced directly in
"T" layout [d_head, seq]. Scores are computed k-major (ST [k, q]) so exp(ST)
tiles serve directly as matmul operands for both the PV product (V stationary)
and the softmax denominator (all-ones stationary -> row-broadcast denominators
in PSUM), with zero on-chip transposes of the attention pattern. Rotary is
applied in a permuted head layout (even dims first) so pair elements sit in
partition halves; W_Q/W_K/b_Q/b_K are pre-permuted on the host to match.
"""

import numpy as np

SEQ = 4096
D_MODEL = 2048
D_HEAD = 128
N_HEADS = 16
N_KV = 4
N_CORES = 8
ROTARY_BASE = 10000.0
ATTN_SCALE = 11.313708498984761  # sqrt(d_head)

P = 128  # partitions
FD = 512  # matmul moving free dim / chunk width


def build_bass(seq=SEQ, d_model=D_MODEL, heads_per_core=2):
    """Emit the per-core Tile kernel. Same program for all cores (SPMD);
    per-core tensors differ only in data."""
    from contextlib import ExitStack

    import concourse.mybir as mybir
    import concourse.tile as tile
    from concourse import bacc
    from concourse.bass import ds

    f32 = mybir.dt.float32
    f32r = mybir.dt.float32r
    AF = mybir.ActivationFunctionType
    OP = mybir.AluOpType

    H = heads_per_core
    DM_TILES = d_model // P      # contraction tiles for projections
    QC = seq // FD               # 512-wide seq chunks
    MC = d_model // FD           # 512-wide output-model chunks

    nc = bacc.Bacc("TRN2", target_bir_lowering=False, debug=False,
                   num_devices=N_CORES)

    xT = nc.dram_tensor("xT", (d_model, seq), f32r, kind="ExternalInput").ap()
    wq = nc.dram_tensor("wq", (H, d_model, D_HEAD), f32r, kind="ExternalInput").ap()
    wk = nc.dram_tensor("wk", (d_model, D_HEAD), f32r, kind="ExternalInput").ap()
    wv = nc.dram_tensor("wv", (d_model, D_HEAD), f32r, kind="ExternalInput").ap()
    wo = nc.dram_tensor("wo", (H, D_HEAD, d_model), f32r, kind="ExternalInput").ap()
    bq = nc.dram_tensor("bq", (64, H, 2), f32, kind="ExternalInput").ap()
    bk = nc.dram_tensor("bk", (64, 2), f32, kind="ExternalInput").ap()
    bv = nc.dram_tensor("bv", (P, 1), f32, kind="ExternalInput").ap()
    cos2 = nc.dram_tensor("cos2", (64, seq), f32, kind="ExternalInput").ap()
    sin2 = nc.dram_tensor("sin2", (64, seq), f32, kind="ExternalInput").ap()
    ident = nc.dram_tensor("ident", (P, P), f32, kind="ExternalInput").ap()
    maskm = nc.dram_tensor("maskm", (P, P), f32r, kind="ExternalInput").ap()
    onesd = nc.dram_tensor("onesd", (P, P), f32r, kind="ExternalInput").ap()
    out = nc.dram_tensor("out", (seq, d_model), f32, kind="ExternalOutput").ap()

    with tile.TileContext(nc) as tc, ExitStack() as ctx:
        const = ctx.enter_context(tc.tile_pool(name="const", bufs=1))
        persist = ctx.enter_context(tc.tile_pool(name="persist", bufs=1))
        xt_pool = ctx.enter_context(tc.tile_pool(name="xt", bufs=17))
        qt_pool = ctx.enter_context(tc.tile_pool(name="qt", bufs=2))
        e_pool = ctx.enter_context(tc.tile_pool(name="e", bufs=3))
        wk_pool = ctx.enter_context(tc.tile_pool(name="wk", bufs=2))
        ps = ctx.enter_context(tc.tile_pool(name="ps", bufs=8, space="PSUM"))

        # ---- constants / weights resident in SBUF ----
        # Weight-chunk DMAs are interleaved with the first chunk's xt loads
        # (inside phase1(0)) so the first projection matmuls start ~2us in.
        wq_sb = const.tile([P, H, DM_TILES, D_HEAD], f32r, tag="wq")
        wk_sb = const.tile([P, DM_TILES, D_HEAD], f32r, tag="wk")
        wv_sb = const.tile([P, DM_TILES, D_HEAD], f32r, tag="wv")
        wq_r = wq.rearrange("h (t p) d -> p h t d", p=P)
        wk_r = wk.rearrange("(t p) d -> p t d", p=P)
        wv_r = wv.rearrange("(t p) d -> p t d", p=P)
        id_sb = const.tile([P, P], f32, tag="id")
        nc.sync.dma_start(id_sb[:], ident)
        mask_sb = const.tile([P, P], f32r, tag="mask")
        nc.sync.dma_start(mask_sb[:], maskm)
        bq_sb = const.tile([64, H, 2], f32, tag="bq")
        nc.sync.dma_start(bq_sb[:], bq)
        bk_sb = const.tile([64, 2], f32, tag="bk")
        nc.sync.dma_start(bk_sb[:], bk)
        bv_sb = const.tile([P, 1], f32, tag="bv")
        nc.sync.dma_start(bv_sb[:], bv)
        ones_sb = const.tile([P, P], f32r, tag="ones")
        nc.sync.dma_start(ones_sb[:], onesd)
        cos_sb = const.tile([64, seq], f32, tag="cos")
        sin_sb = const.tile([64, seq], f32, tag="sin")
        wo_sb = const.tile([P, H, d_model], f32r, tag="wo")

        # K^T (rotated) and V (natural [k, d]) for this core's KV head.
        kt_sb = persist.tile([P, seq], f32r, tag="kt")
        v_sb = persist.tile([P, seq // P, P], f32r, tag="v")

        def rotary_evac(psum, dst, b_ap, qc):
            """dst ([P, FD] slice) = rotary(psum + bias) at positions of chunk qc.

            All DVE products run at partitions 0..63 (PSUM in0 may carry a
            different base partition; two SBUF inputs may not)."""
            sl = ds(qc * FD, FD)
            x1, x2 = psum[0:64, :], psum[64:128, :]
            b1, b2 = b_ap[:, 0:1], b_ap[:, 1:2]
            t1 = wk_pool.tile([64, FD], f32, tag="rot_t1")
            t2 = wk_pool.tile([64, FD], f32, tag="rot_t2")
            t3 = wk_pool.tile([64, FD], f32, tag="rot_t3")
            t4 = wk_pool.tile([64, FD], f32, tag="rot_t4")
            nc.vector.scalar_tensor_tensor(t1[:], x1, b1, cos_sb[:, sl],
                                           op0=OP.add, op1=OP.mult)
            nc.vector.scalar_tensor_tensor(t2[:], x2, b2, sin_sb[:, sl],
                                           op0=OP.add, op1=OP.mult)
            nc.vector.scalar_tensor_tensor(t3[:], x1, b1, sin_sb[:, sl],
                                           op0=OP.add, op1=OP.mult)
            nc.vector.scalar_tensor_tensor(t4[:], x2, b2, cos_sb[:, sl],
                                           op0=OP.add, op1=OP.mult)
            # rot1 = x1 cos - x2 sin ; rot2 = x1 sin + x2 cos
            nc.vector.tensor_sub(dst[0:64, :], t1[:], t2[:])
            nc.vector.tensor_add(dst[64:128, :], t3[:], t4[:])

        def phase1(qc):
            """Q/K/V projections for seq chunk qc (two passes over resident
            xt tiles: Q heads first, then K/V -> only 2 PSUM banks at a
            time); returns the qt tile."""
            xts = [xt_pool.tile([P, FD], f32r, tag="xt", name=f"xt_{qc}_{t}")
                   for t in range(DM_TILES)]
            qp = [ps.tile([P, FD], f32, tag="ps", name=f"qp{h}_{qc}") for h in range(H)]
            for t in range(DM_TILES):
                if qc == 0:
                    nc.sync.dma_start(wq_sb[:, :, t, :], wq_r[:, :, t, :])
                nc.sync.dma_start(xts[t][:], xT[ds(t * P, P), ds(qc * FD, FD)])
                mm = dict(start=(t == 0), stop=(t == DM_TILES - 1))
                for h in range(H):
                    nc.tensor.matmul(qp[h][:], wq_sb[:, h, t, :], xts[t][:], **mm)
            if qc == 0:
                nc.sync.dma_start(cos_sb[:], cos2)
                nc.sync.dma_start(sin_sb[:], sin2)
            qt = qt_pool.tile([P, H, FD], f32r, tag="qt", name=f"qt_{qc}")
            for h in range(H):
                rotary_evac(qp[h], qt[:, h, :], bq_sb[:, h, :], qc)

            kp = ps.tile([P, FD], f32, tag="ps", name=f"kp_{qc}")
            vp = ps.tile([P, FD], f32, tag="ps", name=f"vp_{qc}")
            for t in range(DM_TILES):
                if qc == 0:
                    nc.sync.dma_start(wk_sb[:, t, :], wk_r[:, t, :])
                    nc.sync.dma_start(wv_sb[:, t, :], wv_r[:, t, :])
                mm = dict(start=(t == 0), stop=(t == DM_TILES - 1))
                nc.tensor.matmul(kp[:], wk_sb[:, t, :], xts[t][:], **mm)
                nc.tensor.matmul(vp[:], wv_sb[:, t, :], xts[t][:], **mm)
            if qc == 0:
                nc.sync.dma_start(wo_sb[:], wo.rearrange("h p m -> p h m"))
            rotary_evac(kp, kt_sb[:, ds(qc * FD, FD)], bk_sb, qc)
            # V: bias add then transpose to natural [k, d] layout
            vt = wk_pool.tile([P, FD], f32, tag="vt")
            nc.scalar.activation(vt[:], vp[:], AF.Identity, bias=bv_sb[:, 0:1])
            for j in range(FD // P):
                tp = ps.tile([P, P], f32, tag="ps", name=f"tp_{qc}_{j}")
                nc.tensor.transpose(tp[:], vt[:, ds(j * P, P)], id_sb[:])
                nc.scalar.copy(v_sb[:, qc * (FD // P) + j, :], tp[:])
            return qt

        def attention(qc, qt):
            """Causal attention for q chunk qc; returns per-head normalized z^T."""
            ztn = []
            for h in range(H):
                zt = ps.tile([P, FD], f32, tag="ps", name=f"zt_{h}_{qc}")
                den = ps.tile([P, FD], f32, tag="ps", name=f"den_{h}_{qc}")
                kt_max = 4 * qc + 3
                for kt in range(kt_max + 1):
                    o = max(0, kt * P - qc * FD)
                    n = FD - o
                    st = ps.tile([P, FD], f32, tag="ps", name=f"st_{h}_{qc}_{kt}")
                    nc.tensor.matmul(st[:, o:FD], kt_sb[:, ds(kt * P, P)],
                                     qt[:, h, o:FD], start=True, stop=True)
                    e = e_pool.tile([P, FD], f32r, tag="e", name=f"e_{h}_{qc}_{kt}")
                    nc.scalar.activation(e[:, o:FD], st[:, o:FD], AF.Exp,
                                         scale=1.0 / ATTN_SCALE)
                    if kt >= 4 * qc:  # diagonal 128-block: causal mask inside
                        nc.vector.tensor_mul(e[:, o:o + P], e[:, o:o + P], mask_sb[:])
                    acc = dict(start=(kt == 0), stop=(kt == kt_max))
                    nc.tensor.matmul(zt[:, o:FD], v_sb[:, kt, :], e[:, o:FD], **acc)
                    nc.tensor.matmul(den[0:1, o:FD], ones_sb[:, 0:1], e[:, o:FD], **acc)
                # reciprocal of one denominator row, broadcast via K=1 matmul
                rf = wk_pool.tile([1, FD], f32, tag="rf", bufs=1, name=f"rf_{h}_{qc}")
                nc.vector.reciprocal_approx_fast(rf[:], den[0:1, :])
                rr = wk_pool.tile([1, FD], f32r, tag="rr", bufs=1, name=f"rr_{h}_{qc}")
                nc.vector.tensor_scalar_mul(rr[:], rf[:], 1.0)
                bc = ps.tile([P, FD], f32, tag="ps", name=f"bc_{h}_{qc}")
                nc.tensor.matmul(bc[:], ones_sb[0:1, :], rr[:],
                                 start=True, stop=True)
                rden = wk_pool.tile([P, FD], f32, tag="rden", name=f"rd_{h}_{qc}")
                nc.vector.tensor_copy(rden[:], bc[:])
                z = wk_pool.tile([P, FD], f32r, tag="ztn", bufs=3, name=f"z_{h}_{qc}")
                nc.vector.tensor_mul(z[:], zt[:], rden[:])
                ztn.append(z)
            return ztn

        def outproj(qc, ztn):
            for sub in range(FD // P):
                for mc in range(MC):
                    op_ps = ps.tile([P, FD], f32, tag="ps", name=f"op_{qc}_{sub}_{mc}")
                    for h in range(H):
                        nc.tensor.matmul(op_ps[:], ztn[h][:, ds(sub * P, P)],
                                         wo_sb[:, h, ds(mc * FD, FD)],
                                         start=(h == 0), stop=(h == H - 1))
                    ot = wk_pool.tile([P, FD], f32, tag="ot", bufs=2,
                                      name=f"ot_{qc}_{sub}_{mc}")
                    nc.vector.tensor_copy(ot[:], op_ps[:])
                    nc.sync.dma_start(out[ds(qc * FD + sub * P, P), ds(mc * FD, FD)],
                                      ot[:])

        # Warm-up: dummy matmuls on the ones tile while the first weight/xt
        # DMAs stream in, so the PE clock gate (HAM) is already at full rate
        # when real work starts.
        wup = ps.tile([P, FD], f32, tag="ps", name="warmup_ps")
        for i in range(40):
            nc.tensor.matmul(wup[:, 0:P], ones_sb[:], ones_sb[:],
                             start=(i == 0), stop=(i == 39))

        # Software pipeline: projections for chunk qc+1 are emitted before
        # attention of chunk qc so the PE always has runnable matmuls while
        # attention waits on softmax chains.
        qts = {0: phase1(0)}
        for qc in range(QC):
            if qc + 1 < QC:
                qts[qc + 1] = phase1(qc + 1)
            ztn = attention(qc, qts.pop(qc))
            outproj(qc, ztn)
    nc.compile()
    return nc


_PERM = None


def _perm():
    global _PERM
    if _PERM is None:
        _PERM = np.concatenate([np.arange(0, D_HEAD, 2), np.arange(1, D_HEAD, 2)])
    return _PERM


def host_inputs(x, W_Q, W_K, W_V, W_O, b_Q, b_K, b_V, core,
                heads_per_core=2):
    """Build the per-core input map (numpy, named as in build_bass)."""
    seq = x.shape[0]
    perm = _perm()
    h0 = core * heads_per_core
    kv = h0 // (N_HEADS // N_KV)
    pairs = D_HEAD // 2
    freqs = 1.0 / ROTARY_BASE ** (np.arange(pairs, dtype=np.float64) / pairs)
    ang = np.outer(np.arange(seq), freqs)  # [seq, 64]
    cos = np.cos(ang).T.astype(np.float32)  # [64, seq]
    sin = np.sin(ang).T.astype(np.float32)
    return {
        "xT": np.ascontiguousarray(x.T),
        "wq": np.ascontiguousarray(W_Q[h0:h0 + heads_per_core][:, :, perm]),
        "wk": np.ascontiguousarray(W_K[kv][:, perm]),
        "wv": np.ascontiguousarray(W_V[kv]),
        "wo": np.ascontiguousarray(W_O[h0:h0 + heads_per_core]),
        "bq": np.ascontiguousarray(
            b_Q[h0:h0 + heads_per_core][:, perm]
            .reshape(heads_per_core, 2, 64).transpose(2, 0, 1)),
        "bk": np.ascontiguousarray(b_K[kv][perm].reshape(2, 64).T),
        "bv": np.ascontiguousarray(b_V[kv][:, None]),
        "cos2": cos,
        "sin2": sin,
        "ident": np.eye(P, dtype=np.float32),
        "maskm": np.triu(np.ones((P, P), dtype=np.float32)),
        "onesd": np.ones((P, P), dtype=np.float32),
    }


_NC_CACHE = {}


def kernel(x, W_Q, W_K, W_V, W_O, b_Q, b_K, b_V, b_O):
    import sys
    if "/opt/trn_rl_repo" not in sys.path:
        sys.path.insert(0, "/opt/trn_rl_repo")
    from concourse import bass_utils

    x = np.asarray(x, dtype=np.float32)
    key = (x.shape[0], x.shape[1])
    if key not in _NC_CACHE:
        _NC_CACHE[key] = build_bass(seq=x.shape[0], d_model=x.shape[1])
    nc = _NC_CACHE[key]

    in_maps = [
        host_inputs(x, np.asarray(W_Q, np.float32), np.asarray(W_K, np.float32),
                    np.asarray(W_V, np.float32), np.asarray(W_O, np.float32),
                    np.asarray(b_Q, np.float32), np.asarray(b_K, np.float32),
                    np.asarray(b_V, np.float32), core)
        for core in range(N_CORES)
    ]
    res = bass_utils.run_bass_kernel_spmd(nc, in_maps, core_ids=list(range(N_CORES)))
    total = np.zeros((x.shape[0], x.shape[1]), dtype=np.float32)
    for r in res.results:
        total += r["out"]
    total += np.asarray(b_O, np.float32)[None, :]
    return total


# revision 30
# speedup vs baseline: 1.0140x; 1.0140x over previous
"""Trainium2 Bass kernel for causal GQA attention (nn_Attention_83090437308676).

Full shapes: x [4096, 2048], 16 Q heads / 4 KV heads, d_head=128, fp32, causal,
rotary (interleaved pairs, rotary_dim=128), out = attn @ W_O + b_O.

Sharding: tensor-parallel over heads. Core c computes Q-heads {2c, 2c+1} and
KV-head c//2 (duplicated across the pair of cores sharing it), produces the
partial output z_h @ W_O_h summed over its 2 heads; the host sums the 8
partials and adds b_O.

Device-side layout trick: all matmuls contract on the partition axis, so x is
fed pre-transposed (xT [d_model, seq]) and Q/K are produced directly in
"T" layout [d_head, seq]. Scores are computed k-major (ST [k, q]) so exp(ST)
tiles serve directly as matmul operands for both the PV product (V stationary)
and the softmax denominator (all-ones stationary -> row-broadcast denominators
in PSUM), with zero on-chip transposes of the attention pattern. Rotary is
applied in a permuted head layout (even dims first) so pair elements sit in
partition halves; W_Q/W_K/b_Q/b_K are pre-permuted on the host to match.
"""

import numpy as np

SEQ = 4096
D_MODEL = 2048
D_HEAD = 128
N_HEADS = 16
N_KV = 4
N_CORES = 8
ROTARY_BASE = 10000.0
ATTN_SCALE = 11.313708498984761  # sqrt(d_head)

P = 128  # partitions
FD = 512  # matmul moving free dim / chunk width


def build_bass(seq=SEQ, d_model=D_MODEL, heads_per_core=2):
    """Emit the per-core Tile kernel. Same program for all cores (SPMD);
    per-core tensors differ only in data."""
    from contextlib import ExitStack

    import concourse.mybir as mybir
    import concourse.tile as tile
    from concourse import bacc
    from concourse.bass import ds

    f32 = mybir.dt.float32
    f32r = mybir.dt.float32r
    AF = mybir.ActivationFunctionType
    OP = mybir.AluOpType

    H = heads_per_core
    DM_TILES = d_model // P      # contraction tiles for projections
    QC = seq // FD               # 512-wide seq chunks
    MC = d_model // FD           # 512-wide output-model chunks

    nc = bacc.Bacc("TRN2", target_bir_lowering=False, debug=False,
                   num_devices=N_CORES)

    xT = nc.dram_tensor("xT", (d_model, seq), f32r, kind="ExternalInput").ap()
    wq = nc.dram_tensor("wq", (H, d_model, D_HEAD), f32r, kind="ExternalInput").ap()
    wk = nc.dram_tensor("wk", (d_model, D_HEAD), f32r, kind="ExternalInput").ap()
    wv = nc.dram_tensor("wv", (d_model, D_HEAD), f32r, kind="ExternalInput").ap()
    wo = nc.dram_tensor("wo", (H, D_HEAD, d_model), f32r, kind="ExternalInput").ap()
    bq = nc.dram_tensor("bq", (64, H, 2), f32, kind="ExternalInput").ap()
    bk = nc.dram_tensor("bk", (64, 2), f32, kind="ExternalInput").ap()
    bv = nc.dram_tensor("bv", (P, 1), f32, kind="ExternalInput").ap()
    cos2 = nc.dram_tensor("cos2", (64, seq), f32, kind="ExternalInput").ap()
    sin2 = nc.dram_tensor("sin2", (64, seq), f32, kind="ExternalInput").ap()
    ident = nc.dram_tensor("ident", (P, P), f32, kind="ExternalInput").ap()
    maskm = nc.dram_tensor("maskm", (P, P), f32r, kind="ExternalInput").ap()
    onesd = nc.dram_tensor("onesd", (P, P), f32r, kind="ExternalInput").ap()
    out = nc.dram_tensor("out", (seq, d_model), f32, kind="ExternalOutput").ap()

    with tile.TileContext(nc) as tc, ExitStack() as ctx:
        const = ctx.enter_context(tc.tile_pool(name="const", bufs=1))
        persist = ctx.enter_context(tc.tile_pool(name="persist", bufs=1))
        xt_pool = ctx.enter_context(tc.tile_pool(name="xt", bufs=17))
        qt_pool = ctx.enter_context(tc.tile_pool(name="qt", bufs=3))
        e_pool = ctx.enter_context(tc.tile_pool(name="e", bufs=3))
        wk_pool = ctx.enter_context(tc.tile_pool(name="wk", bufs=2))
        ps = ctx.enter_context(tc.tile_pool(name="ps", bufs=8, space="PSUM"))

        # ---- constants / weights resident in SBUF ----
        # Weight-chunk DMAs are interleaved with the first chunk's xt loads
        # (inside phase1(0)) so the first projection matmuls start ~2us in.
        wq_sb = const.tile([P, H, DM_TILES, D_HEAD], f32r, tag="wq")
        wk_sb = const.tile([P, DM_TILES, D_HEAD], f32r, tag="wk")
        wv_sb = const.tile([P, DM_TILES, D_HEAD], f32r, tag="wv")
        wq_r = wq.rearrange("h (t p) d -> p h t d", p=P)
        wk_r = wk.rearrange("(t p) d -> p t d", p=P)
        wv_r = wv.rearrange("(t p) d -> p t d", p=P)
        id_sb = const.tile([P, P], f32, tag="id")
        nc.sync.dma_start(id_sb[:], ident)
        mask_sb = const.tile([P, P], f32r, tag="mask")
        nc.sync.dma_start(mask_sb[:], maskm)
        bq_sb = const.tile([64, H, 2], f32, tag="bq")
        nc.sync.dma_start(bq_sb[:], bq)
        bk_sb = const.tile([64, 2], f32, tag="bk")
        nc.sync.dma_start(bk_sb[:], bk)
        bv_sb = const.tile([P, 1], f32, tag="bv")
        nc.sync.dma_start(bv_sb[:], bv)
        ones_sb = const.tile([P, P], f32r, tag="ones")
        nc.sync.dma_start(ones_sb[:], onesd)
        cos_sb = const.tile([64, seq], f32, tag="cos")
        sin_sb = const.tile([64, seq], f32, tag="sin")
        wo_sb = const.tile([P, H, d_model], f32r, tag="wo")

        # K^T (rotated) and V (natural [k, d]) for this core's KV head.
        kt_sb = persist.tile([P, seq], f32r, tag="kt")
        v_sb = persist.tile([P, seq // P, P], f32r, tag="v")

        def rotary_evac(psum, dst, b_ap, qc):
            """dst ([P, FD] slice) = rotary(psum + bias) at positions of chunk qc.

            All DVE products run at partitions 0..63 (PSUM in0 may carry a
            different base partition; two SBUF inputs may not)."""
            sl = ds(qc * FD, FD)
            x1, x2 = psum[0:64, :], psum[64:128, :]
            b1, b2 = b_ap[:, 0:1], b_ap[:, 1:2]
            t1 = wk_pool.tile([64, FD], f32, tag="rot_t1")
            t2 = wk_pool.tile([64, FD], f32, tag="rot_t2")
            t3 = wk_pool.tile([64, FD], f32, tag="rot_t3")
            t4 = wk_pool.tile([64, FD], f32, tag="rot_t4")
            nc.vector.scalar_tensor_tensor(t1[:], x1, b1, cos_sb[:, sl],
                                           op0=OP.add, op1=OP.mult)
            nc.vector.scalar_tensor_tensor(t2[:], x2, b2, sin_sb[:, sl],
                                           op0=OP.add, op1=OP.mult)
            nc.vector.scalar_tensor_tensor(t3[:], x1, b1, sin_sb[:, sl],
                                           op0=OP.add, op1=OP.mult)
            nc.vector.scalar_tensor_tensor(t4[:], x2, b2, cos_sb[:, sl],
                                           op0=OP.add, op1=OP.mult)
            # rot1 = x1 cos - x2 sin ; rot2 = x1 sin + x2 cos
            nc.vector.tensor_sub(dst[0:64, :], t1[:], t2[:])
            nc.vector.tensor_add(dst[64:128, :], t3[:], t4[:])

        def phase1(qc):
            """Q/K/V projections for seq chunk qc (two passes over resident
            xt tiles: Q heads first, then K/V -> only 2 PSUM banks at a
            time); returns the qt tile."""
            xts = [xt_pool.tile([P, FD], f32r, tag="xt", name=f"xt_{qc}_{t}")
                   for t in range(DM_TILES)]
            qp = [ps.tile([P, FD], f32, tag="ps", name=f"qp{h}_{qc}") for h in range(H)]
            for t in range(DM_TILES):
                if qc == 0:
                    nc.sync.dma_start(wq_sb[:, :, t, :], wq_r[:, :, t, :])
                nc.sync.dma_start(xts[t][:], xT[ds(t * P, P), ds(qc * FD, FD)])
                mm = dict(start=(t == 0), stop=(t == DM_TILES - 1))
                for h in range(H):
                    nc.tensor.matmul(qp[h][:], wq_sb[:, h, t, :], xts[t][:], **mm)
            if qc == 0:
                nc.sync.dma_start(cos_sb[:], cos2)
                nc.sync.dma_start(sin_sb[:], sin2)
            qt = qt_pool.tile([P, H, FD], f32r, tag="qt", name=f"qt_{qc}")
            for h in range(H):
                rotary_evac(qp[h], qt[:, h, :], bq_sb[:, h, :], qc)

            kp = ps.tile([P, FD], f32, tag="ps", name=f"kp_{qc}")
            vp = ps.tile([P, FD], f32, tag="ps", name=f"vp_{qc}")
            for t in range(DM_TILES):
                if qc == 0:
                    nc.sync.dma_start(wk_sb[:, t, :], wk_r[:, t, :])
                    nc.sync.dma_start(wv_sb[:, t, :], wv_r[:, t, :])
                mm = dict(start=(t == 0), stop=(t == DM_TILES - 1))
                nc.tensor.matmul(kp[:], wk_sb[:, t, :], xts[t][:], **mm)
                nc.tensor.matmul(vp[:], wv_sb[:, t, :], xts[t][:], **mm)
            if qc == 0:
                nc.sync.dma_start(wo_sb[:], wo.rearrange("h p m -> p h m"))
            rotary_evac(kp, kt_sb[:, ds(qc * FD, FD)], bk_sb, qc)
            # V: bias add then transpose to natural [k, d] layout
            vt = wk_pool.tile([P, FD], f32, tag="vt")
            nc.scalar.activation(vt[:], vp[:], AF.Identity, bias=bv_sb[:, 0:1])
            for j in range(FD // P):
                tp = ps.tile([P, P], f32, tag="ps", name=f"tp_{qc}_{j}")
                nc.tensor.transpose(tp[:], vt[:, ds(j * P, P)], id_sb[:])
                nc.scalar.copy(v_sb[:, qc * (FD // P) + j, :], tp[:])
            return qt

        def attention(qc, qt):
            """Causal attention for q chunk qc; returns per-head normalized z^T."""
            ztn = []
            for h in range(H):
                zt = ps.tile([P, FD], f32, tag="ps", name=f"zt_{h}_{qc}")
                den = ps.tile([P, FD], f32, tag="ps", name=f"den_{h}_{qc}")
                kt_max = 4 * qc + 3
                for kt in range(kt_max + 1):
                    o = max(0, kt * P - qc * FD)
                    n = FD - o
                    st = ps.tile([P, FD], f32, tag="ps", name=f"st_{h}_{qc}_{kt}")
                    nc.tensor.matmul(st[:, o:FD], kt_sb[:, ds(kt * P, P)],
                                     qt[:, h, o:FD], start=True, stop=True)
                    e = e_pool.tile([P, FD], f32r, tag="e", name=f"e_{h}_{qc}_{kt}")
                    nc.scalar.activation(e[:, o:FD], st[:, o:FD], AF.Exp,
                                         scale=1.0 / ATTN_SCALE)
                    if kt >= 4 * qc:  # diagonal 128-block: causal mask inside
                        nc.vector.tensor_mul(e[:, o:o + P], e[:, o:o + P], mask_sb[:])
                    acc = dict(start=(kt == 0), stop=(kt == kt_max))
                    nc.tensor.matmul(zt[:, o:FD], v_sb[:, kt, :], e[:, o:FD], **acc)
                    nc.tensor.matmul(den[0:1, o:FD], ones_sb[:, 0:1], e[:, o:FD], **acc)
                # reciprocal of one denominator row, broadcast via K=1 matmul
                rf = wk_pool.tile([1, FD], f32, tag="rf", bufs=1, name=f"rf_{h}_{qc}")
                nc.vector.reciprocal_approx_fast(rf[:], den[0:1, :])
                rr = wk_pool.tile([1, FD], f32r, tag="rr", bufs=1, name=f"rr_{h}_{qc}")
                nc.vector.tensor_scalar_mul(rr[:], rf[:], 1.0)
                bc = ps.tile([P, FD], f32, tag="ps", name=f"bc_{h}_{qc}")
                nc.tensor.matmul(bc[:], ones_sb[0:1, :], rr[:],
                                 start=True, stop=True)
                rden = wk_pool.tile([P, FD], f32, tag="rden", name=f"rd_{h}_{qc}")
                nc.vector.tensor_copy(rden[:], bc[:])
                z = wk_pool.tile([P, FD], f32r, tag="ztn", bufs=3, name=f"z_{h}_{qc}")
                nc.vector.tensor_mul(z[:], zt[:], rden[:])
                ztn.append(z)
            return ztn

        def outproj(qc, ztn):
            for sub in range(FD // P):
                for mc in range(MC):
                    op_ps = ps.tile([P, FD], f32, tag="ps", name=f"op_{qc}_{sub}_{mc}")
                    for h in range(H):
                        nc.tensor.matmul(op_ps[:], ztn[h][:, ds(sub * P, P)],
                                         wo_sb[:, h, ds(mc * FD, FD)],
                                         start=(h == 0), stop=(h == H - 1))
                    ot = wk_pool.tile([P, FD], f32, tag="ot", bufs=2,
                                      name=f"ot_{qc}_{sub}_{mc}")
                    nc.scalar.copy(ot[:], op_ps[:])
                    nc.sync.dma_start(out[ds(qc * FD + sub * P, P), ds(mc * FD, FD)],
                                      ot[:])

        # Software pipeline: projections run up to two chunks ahead of
        # attention; the output projection stays in-iteration.
        qts = {0: phase1(0)}
        if QC > 1:
            qts[1] = phase1(1)
        for qc in range(QC):
            if qc + 2 < QC:
                qts[qc + 2] = phase1(qc + 2)
            ztn = attention(qc, qts.pop(qc))
            outproj(qc, ztn)
    nc.compile()
    return nc


_PERM = None


def _perm():
    global _PERM
    if _PERM is None:
        _PERM = np.concatenate([np.arange(0, D_HEAD, 2), np.arange(1, D_HEAD, 2)])
    return _PERM


def host_inputs(x, W_Q, W_K, W_V, W_O, b_Q, b_K, b_V, core,
                heads_per_core=2):
    """Build the per-core input map (numpy, named as in build_bass)."""
    seq = x.shape[0]
    perm = _perm()
    h0 = core * heads_per_core
    kv = h0 // (N_HEADS // N_KV)
    pairs = D_HEAD // 2
    freqs = 1.0 / ROTARY_BASE ** (np.arange(pairs, dtype=np.float64) / pairs)
    ang = np.outer(np.arange(seq), freqs)  # [seq, 64]
    cos = np.cos(ang).T.astype(np.float32)  # [64, seq]
    sin = np.sin(ang).T.astype(np.float32)
    return {
        "xT": np.ascontiguousarray(x.T),
        "wq": np.ascontiguousarray(W_Q[h0:h0 + heads_per_core][:, :, perm]),
        "wk": np.ascontiguousarray(W_K[kv][:, perm]),
        "wv": np.ascontiguousarray(W_V[kv]),
        "wo": np.ascontiguousarray(W_O[h0:h0 + heads_per_core]),
        "bq": np.ascontiguousarray(
            b_Q[h0:h0 + heads_per_core][:, perm]
            .reshape(heads_per_core, 2, 64).transpose(2, 0, 1)),
        "bk": np.ascontiguousarray(b_K[kv][perm].reshape(2, 64).T),
        "bv": np.ascontiguousarray(b_V[kv][:, None]),
        "cos2": cos,
        "sin2": sin,
        "ident": np.eye(P, dtype=np.float32),
        "maskm": np.triu(np.ones((P, P), dtype=np.float32)),
        "onesd": np.ones((P, P), dtype=np.float32),
    }


_NC_CACHE = {}


def kernel(x, W_Q, W_K, W_V, W_O, b_Q, b_K, b_V, b_O):
    import sys
    if "/opt/trn_rl_repo" not in sys.path:
        sys.path.insert(0, "/opt/trn_rl_repo")
    from concourse import bass_utils

    x = np.asarray(x, dtype=np.float32)
    key = (x.shape[0], x.shape[1])
    if key not in _NC_CACHE:
        _NC_CACHE[key] = build_bass(seq=x.shape[0], d_model=x.shape[1])
    nc = _NC_CACHE[key]

    in_maps = [
        host_inputs(x, np.asarray(W_Q, np.float32), np.asarray(W_K, np.float32),
                    np.asarray(W_V, np.float32), np.asarray(W_O, np.float32),
                    np.asarray(b_Q, np.float32), np.asarray(b_K, np.float32),
                    np.asarray(b_V, np.float32), core)
        for core in range(N_CORES)
    ]
    res = bass_utils.run_bass_kernel_spmd(nc, in_maps, core_ids=list(range(N_CORES)))
    total = np.zeros((x.shape[0], x.shape[1]), dtype=np.float32)
    for r in res.results:
        total += r["out"]
    total += np.asarray(b_O, np.float32)[None, :]
    return total
